# revision 11
# baseline (speedup 1.0000x reference)
"""Trainium2 Bass kernel: GQA causal self-attention with ALiBi + QK-RMSNorm.

Model: B=1, S=2048, DM=4096, H=32 q-heads, HKV=8 kv-heads, HD=128.
Sharding: tensor-parallel over heads across 8 cores. Core g computes
q-heads 4g..4g+3 with kv-head g, and a row-parallel partial of the output
projection; the host sums the 8 partials (the unshard for row-parallel Wo).

Layout strategy (per core):
  - x is passed transposed (XT [DM,S]) so every projection matmul contracts
    over DM on the partition axis with no on-device transposes.
  - Q,K are produced transposed ([d, s]); V natural ([s, d]).
  - RMSNorm over d (= partition axis) uses a ones-vector matmul for the
    per-position sum of squares, then a partition-broadcast of 1/rms.
  - Scores are computed transposed: S^T[j,i] (j=key pos on partitions,
    i=query pos on free axis). With q scaled by 1/sqrt(HD) and RMSNormed,
    |s| <= sqrt(128) and the ALiBi bias slope*(j-i) <= 0 after causal
    masking, so exp() cannot overflow and NO row-max pass is needed.
    exp bias: +slope*j enters via the ACT per-partition bias operand,
    -slope*i via one DVE row add; causal mask is a precomputed [128,128]
    additive -1e30 triangle on diagonal blocks.
  - P^T tiles feed the PV matmul directly as lhsT (no P transposes).
    A ones column appended to V yields the softmax denominators for free.
  - O ([s,d]) is normalized per-partition, then PE-transposed (64 small
    transposes) into O^T for the output projection.
"""

import math

import numpy as np
import ml_dtypes

import concourse.bass as bass
import concourse.bacc as bacc
import concourse.mybir as mybir
import concourse.tile as tile
from concourse.masks import make_identity

F32 = mybir.dt.float32
BF16 = mybir.dt.bfloat16
AF = mybir.ActivationFunctionType
ALU = mybir.AluOpType

B, S, DM = 1, 2048, 4096
H, HKV, HD = 32, 8, 128
NC_CORES = 8
HL = H // NC_CORES          # 4 local q heads per core
EPS = 1e-6
NEG = -1.0e30
P = 128

NBF = ml_dtypes.bfloat16


def _alibi_slopes(n_heads: int) -> np.ndarray:
    start = 2 ** (-(2 ** (-(math.log2(n_heads) - 3))))
    return np.array([start * (start**i) for i in range(n_heads)], dtype=np.float32)


def build_module(s: int = S):
    """Build the per-core Bass module. `s` parameterized for small-scale tests."""
    assert s % 512 == 0
    nss = s // 512            # 512-wide s slices for the projection phase
    njt = s // P              # 128-wide j (key) tiles
    nib = s // 512            # 512-wide i (query) blocks
    ndm = DM // P             # 32 contraction tiles

    nc = bacc.Bacc(trn_type="TRN2")

    xt_d = nc.dram_tensor("xt", [DM, s], BF16, kind="ExternalInput")
    wq_d = nc.dram_tensor("wq", [DM, HL * HD], BF16, kind="ExternalInput")
    wk_d = nc.dram_tensor("wk", [DM, HD], BF16, kind="ExternalInput")
    wv_d = nc.dram_tensor("wv", [DM, HD], BF16, kind="ExternalInput")
    wo_d = nc.dram_tensor("wo", [HL * HD, DM], BF16, kind="ExternalInput")
    qnw_d = nc.dram_tensor("qnw", [HD, 1], F32, kind="ExternalInput")
    knw_d = nc.dram_tensor("knw", [HD, 1], F32, kind="ExternalInput")
    slp_d = nc.dram_tensor("slp", [P, HL], F32, kind="ExternalInput")
    nslp_d = nc.dram_tensor("nslp", [P, HL], F32, kind="ExternalInput")
    out_d = nc.dram_tensor("out", [s, DM], BF16, kind="ExternalOutput")

    with tile.TileContext(nc) as tc:
        with (
            tc.tile_pool(name="const", bufs=1) as const,
            tc.tile_pool(name="xt", bufs=2) as xt_pool,
            tc.tile_pool(name="big", bufs=1) as big,
            tc.tile_pool(name="sq", bufs=2) as sq_pool,
            tc.tile_pool(name="row1", bufs=2) as row1,
            tc.tile_pool(name="inv", bufs=2) as inv_pool,
            tc.tile_pool(name="nrow", bufs=2) as nr_pool,
            tc.tile_pool(name="jcol", bufs=2) as jc_pool,
            tc.tile_pool(name="tmp", bufs=3) as tmp_pool,
            tc.tile_pool(name="pt", bufs=3) as pt_pool,
            tc.tile_pool(name="osb", bufs=4) as osb_pool,
            tc.tile_pool(name="rl", bufs=4) as rl_pool,
            tc.tile_pool(name="wo", bufs=2) as wo_pool,
            tc.tile_pool(name="fsb", bufs=3) as fsb_pool,
            tc.tile_pool(name="ps", bufs=8, space="PSUM") as ps,
            tc.tile_pool(name="dscratch", bufs=4, space="DRAM") as dscratch,
        ):
            # ---------------- constants ----------------
            wq_sb = const.tile([P, ndm, HL * HD], BF16)
            nc.sync.dma_start(wq_sb, wq_d[:, :].rearrange("(o p) m -> p o m", p=P))
            wk_sb = const.tile([P, ndm, HD], BF16)
            nc.sync.dma_start(wk_sb, wk_d[:, :].rearrange("(o p) m -> p o m", p=P))
            wv_sb = const.tile([P, ndm, HD], BF16)
            nc.sync.dma_start(wv_sb, wv_d[:, :].rearrange("(o p) m -> p o m", p=P))
            qnw_sb = const.tile([P, 1], F32)
            nc.sync.dma_start(qnw_sb, qnw_d[:, :])
            knw_sb = const.tile([P, 1], F32)
            nc.sync.dma_start(knw_sb, knw_d[:, :])
            slp_sb = const.tile([P, HL], F32)
            nc.sync.dma_start(slp_sb, slp_d[:, :])
            nslp_sb = const.tile([P, HL], F32)
            nc.sync.dma_start(nslp_sb, nslp_d[:, :])

            ones_sb = const.tile([P, 1], F32)
            nc.vector.memset(ones_sb, 1.0)
            eps_sb = const.tile([P, 1], F32)
            nc.vector.memset(eps_sb, EPS)
            ident = const.tile([P, P], BF16)
            make_identity(nc, ident)

            # iota_row[p, f] = f ; iota_jcol[p, t] = 128*t + p
            iota_row = const.tile([P, 512], F32)
            nc.gpsimd.iota(iota_row, pattern=[[1, 512]], base=0,
                           channel_multiplier=0,
                           allow_small_or_imprecise_dtypes=True)
            iota_jcol = const.tile([P, njt], F32)
            nc.gpsimd.iota(iota_jcol, pattern=[[P, njt]], base=0,
                           channel_multiplier=1,
                           allow_small_or_imprecise_dtypes=True)

            # maskneg[p, f] = 0 where p <= f else -1e30  (additive causal mask
            # for diagonal 128x128 blocks of S^T)
            maskneg = const.tile([P, P], F32)
            nc.gpsimd.memset(maskneg, 0.0)
            # out[p, f] = (f - p) >= 0 ? 0.0 : NEG   (keep j <= i)
            nc.gpsimd.affine_select(
                out=maskneg, in_=maskneg,
                compare_op=ALU.is_ge, fill=NEG,
                base=0, pattern=[[1, P]], channel_multiplier=-1,
            )

            # ---------------- persistent activations ----------------
            qt_sb = big.tile([P, HL, s], BF16)      # Q^T per head  [d, s]
            kt_sb = big.tile([P, s], BF16)          # K^T           [d, s]
            v_sb = big.tile([P, njt, HD + 4], BF16)  # V (+ones col) [s, d+1]
            ot_sb = big.tile([P, HL, s], BF16)      # O^T per head  [d, s]
            nc.vector.memset(v_sb[:, :, HD:HD + 1], 1.0)

            xt_r = xt_d[:, :].rearrange("(o p) t -> p o t", p=P)

            # ---------------- projections + rmsnorm ----------------
            for ss in range(nss):
                s0 = ss * 512
                xt_t = xt_pool.tile([P, ndm, 512], BF16)
                nc.sync.dma_start(xt_t, xt_r[:, :, s0:s0 + 512])

                # Q (4 heads) and K projections: out = W^T.T @ X^T -> [d, s]
                pq = [ps.tile([P, 512], F32, tag="ps", name=f"pq{c}") for c in range(HL)]
                pk = ps.tile([P, 512], F32, tag="ps")
                for o in range(ndm):
                    for c in range(HL):
                        nc.tensor.matmul(
                            pq[c], wq_sb[:, o, c * HD:(c + 1) * HD],
                            xt_t[:, o, :], start=(o == 0), stop=(o == ndm - 1))
                    nc.tensor.matmul(
                        pk, wk_sb[:, o, :], xt_t[:, o, :],
                        start=(o == 0), stop=(o == ndm - 1))

                # rmsnorm: sumsq over d (partitions) via ones-matmul, then
                # 1/sqrt bcast back over partitions; fused scale+cast to bf16.
                for c in range(HL + 1):
                    src = pq[c] if c < HL else pk
                    w_sb = qnw_sb if c < HL else knw_sb
                    sqt = sq_pool.tile([P, 512], F32, tag="sq")
                    nc.scalar.activation(sqt, src, AF.Square)
                    psq = ps.tile([1, 512], F32, tag="ps")
                    nc.tensor.matmul(psq, ones_sb, sqt, start=True, stop=True)
                    rms = row1.tile([1, 512], F32, tag="row1")
                    nc.scalar.activation(rms, psq, AF.Sqrt,
                                         bias=eps_sb[:1, :], scale=1.0 / HD)
                    rec = row1.tile([1, 512], F32, tag="row1")
                    nc.vector.reciprocal(rec, rms)
                    rec_d = dscratch.tile([1, 512], F32, tag="recd")
                    nc.sync.dma_start(rec_d, rec)
                    invb = inv_pool.tile([P, 512], F32, tag="inv")
                    rec_bcast = bass.AP(
                        tensor=rec_d.tensor, offset=rec_d.offset,
                        ap=[[0, P], [1, 512]])
                    nc.sync.dma_start(invb, rec_bcast)
                    # out = (src * w) * invb   (w folds 1/sqrt(HD) for q)
                    dst = qt_sb[:, c, s0:s0 + 512] if c < HL \
                        else kt_sb[:, s0:s0 + 512]
                    nc.vector.scalar_tensor_tensor(
                        out=dst, in0=src, scalar=w_sb, in1=invb,
                        op0=ALU.mult, op1=ALU.mult)

                # V projection: out = X^T.T @ Wv^T -> [s, d]
                for c in range(4):
                    pv = ps.tile([P, HD], F32, tag="ps")
                    for o in range(ndm):
                        nc.tensor.matmul(
                            pv, xt_t[:, o, c * P:(c + 1) * P], wv_sb[:, o, :],
                            start=(o == 0), stop=(o == ndm - 1))
                    nc.scalar.copy(v_sb[:, 4 * ss + c, :HD], pv)

            # ---------------- attention ----------------
            for ib in range(nib):
                i0 = ib * 512
                for h in range(HL):
                    negrow = nr_pool.tile([P, 512], F32, tag="nrow")
                    nc.vector.tensor_tensor(
                        negrow, iota_row,
                        nslp_sb[:, h:h + 1].to_broadcast([P, 512]), ALU.mult)
                    # fold the constant -slope*i0 into the exp bias column:
                    # bias[p, jt] = slope * (128*jt + p - i0)
                    jtmp = jc_pool.tile([P, njt], F32, tag="jcol", name="jtmp")
                    nc.vector.tensor_scalar_add(jtmp, iota_jcol, float(-i0))
                    jcol = jc_pool.tile([P, njt], F32, tag="jcol")
                    nc.vector.tensor_tensor(
                        jcol, jtmp,
                        slp_sb[:, h:h + 1].to_broadcast([P, njt]), ALU.mult)

                    ops = [ps.tile([P, HD + 1], F32, tag="ps", name=f"ops{c}")
                           for c in range(4)]
                    for jt in range(4 * (ib + 1)):
                        j0 = jt * P
                        c0 = max(0, j0 - i0)
                        st = ps.tile([P, 512], F32, tag="ps")
                        nc.tensor.matmul(
                            st[:, c0:], kt_sb[:, j0:j0 + P],
                            qt_sb[:, h, i0 + c0:i0 + 512],
                            start=True, stop=True)
                        tmp = tmp_pool.tile([P, 512], F32, tag="tmp")
                        nc.vector.tensor_tensor(
                            tmp[:, c0:], st[:, c0:], negrow[:, c0:], ALU.add)
                        if j0 >= i0:  # diagonal block: additive causal mask
                            nc.vector.tensor_tensor(
                                tmp[:, c0:c0 + P], tmp[:, c0:c0 + P],
                                maskneg, ALU.add)
                        pt = pt_pool.tile([P, 512], BF16, tag="pt")
                        nc.scalar.activation(
                            pt[:, c0:], tmp[:, c0:], AF.Exp,
                            bias=jcol[:, jt:jt + 1], scale=1.0)
                        for c in range(c0 // P, 4):
                            nc.tensor.matmul(
                                ops[c], pt[:, c * P:(c + 1) * P],
                                v_sb[:, jt, :HD + 1],
                                start=(jt == 0), stop=(jt == 4 * ib + c))

                    for c in range(4):
                        rl = rl_pool.tile([P, 1], F32, tag="rl")
                        nc.vector.reciprocal(rl, ops[c][:, HD:HD + 1])
                        osb = osb_pool.tile([P, P], BF16, tag="osb")
                        nc.vector.tensor_tensor(
                            osb, ops[c][:, :HD],
                            rl.to_broadcast([P, HD]), ALU.mult)
                        trp = ps.tile([P, P], BF16, tag="ps")
                        nc.tensor.transpose(trp, osb, ident)
                        nc.vector.tensor_copy(
                            ot_sb[:, h, i0 + c * P:i0 + (c + 1) * P], trp)

            # ---------------- output projection ----------------
            wo_r = wo_d[:, :].rearrange("(o p) m -> p o m", p=P)
            for mi in range(DM // 512):
                m0 = mi * 512
                wo_t = wo_pool.tile([P, HL, 512], BF16, tag="wo")
                nc.sync.dma_start(wo_t, wo_r[:, :, m0:m0 + 512])
                for st_i in range(s // P):
                    s0 = st_i * P
                    fps = ps.tile([P, 512], F32, tag="ps")
                    for c in range(HL):
                        nc.tensor.matmul(
                            fps, ot_sb[:, c, s0:s0 + P], wo_t[:, c, :],
                            start=(c == 0), stop=(c == HL - 1))
                    fsb = fsb_pool.tile([P, 512], BF16, tag="fsb")
                    nc.scalar.copy(fsb, fps)
                    nc.sync.dma_start(out_d[s0:s0 + P, m0:m0 + 512], fsb)

    nc.finalize()
    return nc


def shard_inputs(x, Wq, Wk, Wv, Wo, q_norm_w, k_norm_w, s=S):
    """Host-side shard + layout prep. Returns per-core input maps."""
    slopes = _alibi_slopes(H)
    xt = np.ascontiguousarray(x.reshape(s, DM).T).astype(NBF)
    qnw = (np.asarray(q_norm_w, np.float32) / math.sqrt(HD)).reshape(HD, 1)
    knw = np.asarray(k_norm_w, np.float32).reshape(HD, 1).copy()
    in_maps = []
    for g in range(NC_CORES):
        qs = g * HL * HD
        sl = slopes[g * HL:(g + 1) * HL]
        in_maps.append({
            "xt": xt,
            "wq": np.ascontiguousarray(Wq[qs:qs + HL * HD, :].T).astype(NBF),
            "wk": np.ascontiguousarray(Wk[g * HD:(g + 1) * HD, :].T).astype(NBF),
            "wv": np.ascontiguousarray(Wv[g * HD:(g + 1) * HD, :].T).astype(NBF),
            "wo": np.ascontiguousarray(Wo[:, qs:qs + HL * HD].T).astype(NBF),
            "qnw": qnw,
            "knw": knw,
            "slp": np.ascontiguousarray(
                np.broadcast_to(sl, (P, HL))).astype(np.float32),
            "nslp": np.ascontiguousarray(
                np.broadcast_to(-sl, (P, HL))).astype(np.float32),
        })
    return in_maps


_MODULE_CACHE = {}
LAST_RESULT = None


def _get_module(s=S):
    if s not in _MODULE_CACHE:
        _MODULE_CACHE[s] = build_module(s)
    return _MODULE_CACHE[s]


def kernel(x, Wq, Wk, Wv, Wo, q_norm_w, k_norm_w, **run_kwargs):
    global LAST_RESULT
    from concourse.bass_utils import run_bass_kernel_spmd

    x = np.asarray(x)
    in_maps = shard_inputs(np.asarray(x), np.asarray(Wq), np.asarray(Wk),
                           np.asarray(Wv), np.asarray(Wo),
                           np.asarray(q_norm_w), np.asarray(k_norm_w))
    nc = _get_module(S)
    res = run_bass_kernel_spmd(nc, in_maps, core_ids=list(range(NC_CORES)),
                               **run_kwargs)
    LAST_RESULT = res
    acc = np.zeros((S, DM), np.float32)
    for r in res.results:
        acc += r["out"].astype(np.float32)
    return acc.reshape(B, S, DM)


# revision 22
# speedup vs baseline: 1.6147x; 1.6147x over previous
"""Trainium2 Bass kernel: GQA causal self-attention with ALiBi + QK-RMSNorm.

Model: B=1, S=2048, DM=4096, H=32 q-heads, HKV=8 kv-heads, HD=128.
Sharding: tensor-parallel over heads across 8 cores. Core g computes
q-heads 4g..4g+3 with kv-head g, and a row-parallel partial of the output
projection; the host sums the 8 partials (the unshard for row-parallel Wo).

Layout strategy (per core):
  - x is passed transposed (XT [DM,S]) so every projection matmul contracts
    over DM on the partition axis with no on-device transposes.
  - Q,K are produced transposed ([d, s]); V natural ([s, d]).
  - RMSNorm over d (= partition axis) uses a ones-vector matmul for the
    per-position sum of squares, then a GPSIMD partition_broadcast of 1/rms.
  - Scores are computed transposed: S^T[j,i] (j=key pos on partitions,
    i=query pos on free axis). With q scaled by 1/sqrt(HD) and RMSNormed,
    |s| <= sqrt(128) and the ALiBi bias slope*(j-i) <= 0 after causal
    masking, so exp() cannot overflow and NO row-max pass is needed.
    exp bias: +slope*(j-i0) enters via the ACT per-partition bias operand,
    -slope*(i-i0) via one DVE row add; the causal mask is a precomputed
    [128,128] additive -1e30 triangle applied to diagonal blocks (GPSIMD).
  - P^T tiles feed the PV matmul as rhs with V as lhsT, accumulating O^T
    [d, i] directly in PSUM (no transposes anywhere). A ones-lhsT matmul
    accumulates the softmax denominators as a row, normalized via
    reciprocal + partition_broadcast.
  - Phases are software-pipelined: attention for query block ib runs right
    after projection slice ib, and the (PE-dense) output projection of
    block ib-1 is interleaved into the (dependency-chain-bound) attention
    of block ib to keep the PE fed.
"""

import math

import numpy as np
import ml_dtypes

import concourse.bass as bass
import concourse.bacc as bacc
import concourse.mybir as mybir
import concourse.tile as tile

F32 = mybir.dt.float32
BF16 = mybir.dt.bfloat16
AF = mybir.ActivationFunctionType
ALU = mybir.AluOpType

B, S, DM = 1, 2048, 4096
H, HKV, HD = 32, 8, 128
NC_CORES = 8
HL = H // NC_CORES          # 4 local q heads per core
EPS = 1e-6
NEG = -1.0e30
P = 128

NBF = ml_dtypes.bfloat16


def _alibi_slopes(n_heads: int) -> np.ndarray:
    start = 2 ** (-(2 ** (-(math.log2(n_heads) - 3))))
    return np.array([start * (start**i) for i in range(n_heads)], dtype=np.float32)


def build_module(s: int = S, repeat: int = 1, phases=('proj', 'attn', 'out')):
    """Build the per-core Bass module. `s` parameterized for small tests."""
    assert s % 512 == 0
    nss = s // 512            # 512-wide s slices / query blocks
    njt = s // P              # 128-wide key tiles
    ndm = DM // P             # 32 contraction tiles

    nc = bacc.Bacc(trn_type="TRN2")

    xt_d = nc.dram_tensor("xt", [DM, s], BF16, kind="ExternalInput")
    wq_d = nc.dram_tensor("wq", [DM, HL * HD], BF16, kind="ExternalInput")
    wk_d = nc.dram_tensor("wk", [DM, HD], BF16, kind="ExternalInput")
    wv_d = nc.dram_tensor("wv", [DM, HD], BF16, kind="ExternalInput")
    wo_d = nc.dram_tensor("wo", [HL * HD, DM], BF16, kind="ExternalInput")
    qnw_d = nc.dram_tensor("qnw", [HD, 1], F32, kind="ExternalInput")
    knw_d = nc.dram_tensor("knw", [HD, 1], F32, kind="ExternalInput")
    slp_d = nc.dram_tensor("slp", [P, HL], F32, kind="ExternalInput")
    nslp_d = nc.dram_tensor("nslp", [P, HL], F32, kind="ExternalInput")
    out_d = nc.dram_tensor("out", [s, DM], BF16, kind="ExternalOutput")

    with tile.TileContext(nc) as tc:
        with (
            tc.tile_pool(name="const", bufs=1) as const,
            tc.tile_pool(name="xt", bufs=2) as xt_pool,
            tc.tile_pool(name="big", bufs=1) as big,
            tc.tile_pool(name="sq", bufs=2) as sq_pool,
            tc.tile_pool(name="row1", bufs=3) as row1,
            tc.tile_pool(name="inv", bufs=2) as inv_pool,
            tc.tile_pool(name="nrow", bufs=2) as nr_pool,
            tc.tile_pool(name="jcol", bufs=2) as jc_pool,
            tc.tile_pool(name="tmp", bufs=3) as tmp_pool,
            tc.tile_pool(name="pt", bufs=3) as pt_pool,
            tc.tile_pool(name="fsb", bufs=3) as fsb_pool,
            tc.tile_pool(name="ps", bufs=8, space="PSUM") as ps,
        ):
            # ---------------- constants ----------------
            wq_sb = const.tile([P, ndm, HL * HD], BF16)
            wq_r = wq_d[:, :].rearrange("(o p) m -> p o m", p=P)
            nc.sync.dma_start(wq_sb[:, 0:ndm // 4, :], wq_r[:, 0:ndm // 4, :])
            wk_sb = const.tile([P, ndm, HD], BF16)
            nc.sync.dma_start(wk_sb, wk_d[:, :].rearrange("(o p) m -> p o m", p=P))
            wv_sb = const.tile([P, ndm, HD], BF16)
            wo_sb = const.tile([P, HL, DM], BF16)
            wo_r = wo_d[:, :].rearrange("(o p) m -> p o m", p=P)
            qnw_sb = const.tile([P, 1], F32)
            knw_sb = const.tile([P, 1], F32)
            slp_sb = const.tile([P, HL], F32)
            nslp_sb = const.tile([P, HL], F32)

            def deferred_const_loads():
                # Emitted after proj(0)'s first xt chunks: everything here is
                # first needed tens of microseconds into the kernel.
                for ch in range(1, 4):
                    o0 = ch * (ndm // 4)
                    nc.sync.dma_start(wq_sb[:, o0:o0 + ndm // 4, :],
                                      wq_r[:, o0:o0 + ndm // 4, :])
                nc.sync.dma_start(
                    wv_sb, wv_d[:, :].rearrange("(o p) m -> p o m", p=P))
                nc.sync.dma_start(qnw_sb, qnw_d[:, :])
                nc.sync.dma_start(knw_sb, knw_d[:, :])
                nc.sync.dma_start(slp_sb, slp_d[:, :])
                nc.sync.dma_start(nslp_sb, nslp_d[:, :])

            ones_f32 = const.tile([P, 1], F32)
            nc.vector.memset(ones_f32, 1.0)
            ones_sb = const.tile([P, 1], mybir.dt.float32r)
            nc.scalar.copy(ones_sb, ones_f32)
            ones_bf = const.tile([P, 1], BF16)
            nc.vector.memset(ones_bf, 1.0)
            eps_sb = const.tile([P, 1], F32)
            nc.vector.memset(eps_sb, EPS)

            # iota_row[p, f] = f ; iota_jcol[p, t] = 128*t + p
            iota_row = const.tile([P, 512], F32)
            nc.gpsimd.iota(iota_row, pattern=[[1, 512]], base=0,
                           channel_multiplier=0,
                           allow_small_or_imprecise_dtypes=True)
            iota_jcol = const.tile([P, njt], F32)
            nc.gpsimd.iota(iota_jcol, pattern=[[P, njt]], base=0,
                           channel_multiplier=1,
                           allow_small_or_imprecise_dtypes=True)

            # maskneg[p, f] = 0 where p <= f else -1e30  (additive causal
            # mask for diagonal 128x128 blocks of S^T)
            maskneg = const.tile([P, P], F32)
            nc.gpsimd.memset(maskneg, 0.0)
            nc.gpsimd.affine_select(
                out=maskneg, in_=maskneg,
                compare_op=ALU.is_ge, fill=NEG,
                base=0, pattern=[[1, P]], channel_multiplier=-1,
            )

            # ---------------- persistent activations ----------------
            # qt/ot hold only 2 query blocks (ring): the pipeline uses
            # qt of block ss right after proj(ss), and outproj consumes
            # ot of block ss-1 during attention of block ss.
            qt_sb = big.tile([P, HL, 2, 512], BF16)  # Q^T ring [d, h, ss%2, i]
            kt_sb = big.tile([P, s], BF16)           # K^T      [d, s]
            v_sb = big.tile([P, njt, HD], BF16)      # V        [s, d]
            ot_sb = big.tile([P, HL, 2, 512], BF16)  # O^T ring [d, h, ib%2, i]

            xt_r = xt_d[:, :].rearrange("(o p) t -> p o t", p=P)

            def proj_slice(ss):
                s0 = ss * 512
                xt_t = xt_pool.tile([P, ndm, 512], BF16, name="xt_t")
                for ch in range(4):
                    o0 = ch * (ndm // 4)
                    nc.sync.dma_start(xt_t[:, o0:o0 + ndm // 4, :],
                                      xt_r[:, o0:o0 + ndm // 4, s0:s0 + 512])
                if ss == 0:
                    deferred_const_loads()

                # Q (4 heads) and K projections: [d, s] (transposed)
                pq = [ps.tile([P, 512], F32, tag="ps", name=f"pq{c}")
                      for c in range(HL)]
                pk = ps.tile([P, 512], F32, tag="ps", name="pk")
                for o in range(ndm):
                    for c in range(HL):
                        nc.tensor.matmul(
                            pq[c], wq_sb[:, o, c * HD:(c + 1) * HD],
                            xt_t[:, o, :], start=(o == 0), stop=(o == ndm - 1))
                    nc.tensor.matmul(
                        pk, wk_sb[:, o, :], xt_t[:, o, :],
                        start=(o == 0), stop=(o == ndm - 1))

                # rmsnorm over d (partitions): ones-matmul sumsq -> rsqrt ->
                # partition_broadcast; fused scale+cast to bf16 on evict.
                for c in range(HL + 1):
                    src = pq[c] if c < HL else pk
                    w_sb = qnw_sb if c < HL else knw_sb
                    sqt = sq_pool.tile([P, 512], mybir.dt.float32r,
                                       tag="sq", name="sqt")
                    nc.scalar.activation(sqt, src, AF.Square)
                    psq = ps.tile([1, 512], F32, tag="ps", name="psq")
                    nc.tensor.matmul(psq, ones_sb, sqt,
                                     start=True, stop=True)
                    rms = row1.tile([1, 512], F32, tag="row1", name="rms")
                    nc.scalar.activation(rms, psq, AF.Sqrt,
                                         bias=eps_sb[:1, :], scale=1.0 / HD)
                    rec = row1.tile([1, 512], F32, tag="row1", name="rec")
                    nc.vector.reciprocal(rec, rms)
                    invb = inv_pool.tile([P, 512], F32, tag="inv", name="invb")
                    nc.gpsimd.partition_broadcast(invb, rec)
                    dst = qt_sb[:, c, ss % 2, :] if c < HL \
                        else kt_sb[:, s0:s0 + 512]
                    nc.vector.scalar_tensor_tensor(
                        out=dst, in0=src, scalar=w_sb, in1=invb,
                        op0=ALU.mult, op1=ALU.mult)

                # V projection: [s, d] natural
                for c in range(4):
                    pv = ps.tile([P, HD], F32, tag="ps", name="pv")
                    for o in range(ndm):
                        nc.tensor.matmul(
                            pv, xt_t[:, o, c * P:(c + 1) * P], wv_sb[:, o, :],
                            start=(o == 0), stop=(o == ndm - 1))
                    nc.scalar.copy(v_sb[:, 4 * ss + c, :], pv)

            def outproj_chunk(ib, mi_list):
                """Output projection for query block ib, m-slices mi_list."""
                for mi in mi_list:
                    m0 = mi * 512
                    for st_i in range(4):
                        s0 = ib * 512 + st_i * P
                        fps = ps.tile([P, 512], F32, tag="ps", name="fps")
                        for c in range(HL):
                            nc.tensor.matmul(
                                fps, ot_sb[:, c, ib % 2, st_i * P:(st_i + 1) * P],
                                wo_sb[:, c, m0:m0 + 512],
                                start=(c == 0), stop=(c == HL - 1))
                        fsb = fsb_pool.tile([P, 512], BF16, tag="fsb",
                                            name="fsb")
                        if (mi + st_i) % 2 == 0:
                            nc.scalar.copy(fsb, fps)
                        else:
                            nc.vector.tensor_copy(fsb, fps)
                        nc.sync.dma_start(out_d[s0:s0 + P, m0:m0 + 512], fsb)

            def attn_setup(ib, h):
                i0 = ib * 512
                negrow = nr_pool.tile([P, 512], F32, tag="nrow", name="negrow")
                nc.gpsimd.tensor_tensor(
                    negrow, iota_row,
                    nslp_sb[:, h:h + 1].to_broadcast([P, 512]), ALU.mult)
                # exp bias column: bias[p, jt] = slope * (128*jt + p - i0)
                jtmp = jc_pool.tile([P, njt], F32, tag="jcol", name="jtmp")
                nc.gpsimd.tensor_scalar_add(jtmp, iota_jcol, float(-i0))
                jcol = jc_pool.tile([P, njt], F32, tag="jcol", name="jcol")
                nc.gpsimd.tensor_tensor(
                    jcol, jtmp,
                    slp_sb[:, h:h + 1].to_broadcast([P, njt]), ALU.mult)
                otp = ps.tile([P, 512], F32, tag="ps", name="otp")
                lps = ps.tile([1, 512], F32, tag="ps", name="lps")
                return negrow, jcol, otp, lps

            def attn_jt(ib, h, jt, negrow, jcol, otp, lps):
                i0 = ib * 512
                jlast = 4 * (ib + 1) - 1
                j0 = jt * P
                c0 = max(0, j0 - i0)
                st = ps.tile([P, 512], F32, tag="ps", name="st")
                nc.tensor.matmul(
                    st[:, c0:], kt_sb[:, j0:j0 + P],
                    qt_sb[:, h, ib % 2, c0:],
                    start=True, stop=True)
                tmp = tmp_pool.tile([P, 512], F32, tag="tmp", name="tmp")
                nc.vector.tensor_tensor(
                    tmp[:, c0:], st[:, c0:], negrow[:, c0:], ALU.add)
                if j0 >= i0:  # diagonal block: additive causal mask
                    nc.gpsimd.tensor_tensor(
                        tmp[:, c0:c0 + P], tmp[:, c0:c0 + P],
                        maskneg, ALU.add)
                pt = pt_pool.tile([P, 512], BF16, tag="pt", name="pt")
                nc.scalar.activation(
                    pt[:, c0:], tmp[:, c0:], AF.Exp,
                    bias=jcol[:, jt:jt + 1], scale=1.0)
                # O^T accumulation: otp[d, i] += sum_j V[j, d] P^T[j, i]
                nc.tensor.matmul(
                    otp[:, c0:], v_sb[:, jt, :], pt[:, c0:],
                    start=(jt == 0), stop=(jt == jlast))
                # denominators: lps[0, i] += sum_j P^T[j, i]
                nc.tensor.matmul(
                    lps[:, c0:], ones_bf, pt[:, c0:],
                    start=(jt == 0), stop=(jt == jlast))

            def attn_finish(ib, h, otp, lps):
                lrow = row1.tile([1, 512], F32, tag="row1", name="lrow")
                nc.scalar.copy(lrow, lps)
                linv = row1.tile([1, 512], F32, tag="row1", name="linv")
                nc.vector.reciprocal(linv, lrow)
                linvb = inv_pool.tile([P, 512], F32, tag="inv", name="linvb")
                nc.gpsimd.partition_broadcast(linvb, linv)
                nc.vector.tensor_tensor(
                    ot_sb[:, h, ib % 2, :], otp, linvb, ALU.mult)

            def attn_head(ib, h):
                negrow, jcol, otp, lps = attn_setup(ib, h)
                for jt in range(4 * (ib + 1)):
                    attn_jt(ib, h, jt, negrow, jcol, otp, lps)
                attn_finish(ib, h, otp, lps)

            def attn_head_pair(ib, h0, h1):
                ctx0 = attn_setup(ib, h0)
                ctx1 = attn_setup(ib, h1)
                for jt in range(4 * (ib + 1)):
                    attn_jt(ib, h0, jt, *ctx0)
                    attn_jt(ib, h1, jt, *ctx1)
                attn_finish(ib, h0, ctx0[2], ctx0[3])
                attn_finish(ib, h1, ctx1[2], ctx1[3])

            for _rep in range(repeat):
                # pipelined: proj(ss) -> attention(ss) with outproj(ss-1)
                # interleaved at head granularity to fill PE bubbles.
                for ss in range(nss):
                    if 'proj' in phases:
                        proj_slice(ss)
                    if 'attn' in phases:
                        if ss == 0:
                            for h in range(HL):
                                attn_head(ss, h)
                                if _rep == 0:
                                    # Wo not needed until block 1: load it
                                    # in attention block 0's PE bubbles.
                                    m0 = h * (DM // 4)
                                    nc.sync.dma_start(
                                        wo_sb[:, :, m0:m0 + DM // 4],
                                        wo_r[:, :, m0:m0 + DM // 4])
                        else:
                            for h in range(HL):
                                attn_head(ss, h)
                                if 'out' in phases:
                                    outproj_chunk(ss - 1, [2 * h, 2 * h + 1])
                if 'out' in phases and 'attn' in phases:
                    outproj_chunk(nss - 1, list(range(8)))

    nc.finalize()
    return nc


def shard_inputs(x, Wq, Wk, Wv, Wo, q_norm_w, k_norm_w, s=S):
    """Host-side shard + layout prep. Returns per-core input maps."""
    slopes = _alibi_slopes(H)
    xt = np.ascontiguousarray(x.reshape(s, DM).T).astype(NBF)
    qnw = (np.asarray(q_norm_w, np.float32) / math.sqrt(HD)).reshape(HD, 1)
    knw = np.asarray(k_norm_w, np.float32).reshape(HD, 1).copy()
    in_maps = []
    for g in range(NC_CORES):
        qs = g * HL * HD
        sl = slopes[g * HL:(g + 1) * HL]
        in_maps.append({
            "xt": xt,
            "wq": np.ascontiguousarray(Wq[qs:qs + HL * HD, :].T).astype(NBF),
            "wk": np.ascontiguousarray(Wk[g * HD:(g + 1) * HD, :].T).astype(NBF),
            "wv": np.ascontiguousarray(Wv[g * HD:(g + 1) * HD, :].T).astype(NBF),
            "wo": np.ascontiguousarray(Wo[:, qs:qs + HL * HD].T).astype(NBF),
            "qnw": qnw,
            "knw": knw,
            "slp": np.ascontiguousarray(
                np.broadcast_to(sl, (P, HL))).astype(np.float32),
            "nslp": np.ascontiguousarray(
                np.broadcast_to(-sl, (P, HL))).astype(np.float32),
        })
    return in_maps


_MODULE_CACHE = {}
LAST_RESULT = None


def _get_module(s=S):
    if s not in _MODULE_CACHE:
        _MODULE_CACHE[s] = build_module(s)
    return _MODULE_CACHE[s]


def kernel(x, Wq, Wk, Wv, Wo, q_norm_w, k_norm_w, **run_kwargs):
    global LAST_RESULT
    from concourse.bass_utils import run_bass_kernel_spmd

    x = np.asarray(x)
    in_maps = shard_inputs(np.asarray(x), np.asarray(Wq), np.asarray(Wk),
                           np.asarray(Wv), np.asarray(Wo),
                           np.asarray(q_norm_w), np.asarray(k_norm_w))
    nc = _get_module(S)
    res = run_bass_kernel_spmd(nc, in_maps, core_ids=list(range(NC_CORES)),
                               **run_kwargs)
    LAST_RESULT = res
    acc = np.zeros((S, DM), np.float32)
    for r in res.results:
        acc += r["out"].astype(np.float32)
    return acc.reshape(B, S, DM)


# revision 23
# speedup vs baseline: 68701.3394x; 42546.7987x over previous
"""Trainium2 Bass kernel: GQA causal self-attention with ALiBi + QK-RMSNorm.

Model: B=1, S=2048, DM=4096, H=32 q-heads, HKV=8 kv-heads, HD=128.
Sharding: tensor-parallel over heads across 8 cores. Core g computes
q-heads 4g..4g+3 with kv-head g, and a row-parallel partial of the output
projection; the host sums the 8 partials (the unshard for row-parallel Wo).

Layout strategy (per core):
  - x is passed transposed (XT [DM,S]) so every projection matmul contracts
    over DM on the partition axis with no on-device transposes.
  - Q,K are produced transposed ([d, s]); V natural ([s, d]).
  - RMSNorm over d (= partition axis) uses a ones-vector matmul for the
    per-position sum of squares, then a GPSIMD partition_broadcast of 1/rms.
  - Scores are computed transposed: S^T[j,i] (j=key pos on partitions,
    i=query pos on free axis). With q scaled by 1/sqrt(HD) and RMSNormed,
    |s| <= sqrt(128) and the ALiBi bias slope*(j-i) <= 0 after causal
    masking, so exp() cannot overflow and NO row-max pass is needed.
    exp bias: +slope*(j-i0) enters via the ACT per-partition bias operand,
    -slope*(i-i0) via one DVE row add; the causal mask is a precomputed
    [128,128] additive -1e30 triangle applied to diagonal blocks (GPSIMD).
  - P^T tiles feed the PV matmul as rhs with V as lhsT, accumulating O^T
    [d, i] directly in PSUM (no transposes anywhere). A ones-lhsT matmul
    accumulates the softmax denominators as a row, normalized via
    reciprocal + partition_broadcast.
  - Phases are software-pipelined: attention for query block ib runs right
    after projection slice ib, and the (PE-dense) output projection of
    block ib-1 is interleaved into the (dependency-chain-bound) attention
    of block ib to keep the PE fed.
"""

import math

import numpy as np
import ml_dtypes

import concourse.bass as bass
import concourse.bacc as bacc
import concourse.mybir as mybir
import concourse.tile as tile

F32 = mybir.dt.float32
BF16 = mybir.dt.bfloat16
AF = mybir.ActivationFunctionType
ALU = mybir.AluOpType

B, S, DM = 1, 2048, 4096
H, HKV, HD = 32, 8, 128
NC_CORES = 8
HL = H // NC_CORES          # 4 local q heads per core
EPS = 1e-6
NEG = -1.0e30
P = 128

NBF = ml_dtypes.bfloat16


def _alibi_slopes(n_heads: int) -> np.ndarray:
    start = 2 ** (-(2 ** (-(math.log2(n_heads) - 3))))
    return np.array([start * (start**i) for i in range(n_heads)], dtype=np.float32)


def build_module(s: int = S, repeat: int = 1, phases=('proj', 'attn', 'out')):
    """Build the per-core Bass module. `s` parameterized for small tests."""
    assert s % 512 == 0
    nss = s // 512            # 512-wide s slices / query blocks
    njt = s // P              # 128-wide key tiles
    ndm = DM // P             # 32 contraction tiles

    nc = bacc.Bacc(trn_type="TRN2")

    xt_d = nc.dram_tensor("xt", [DM, s], BF16, kind="ExternalInput")
    wq_d = nc.dram_tensor("wq", [DM, HL * HD], BF16, kind="ExternalInput")
    wk_d = nc.dram_tensor("wk", [DM, HD], BF16, kind="ExternalInput")
    wv_d = nc.dram_tensor("wv", [DM, HD], BF16, kind="ExternalInput")
    wo_d = nc.dram_tensor("wo", [HL * HD, DM], BF16, kind="ExternalInput")
    qnw_d = nc.dram_tensor("qnw", [HD, 1], F32, kind="ExternalInput")
    knw_d = nc.dram_tensor("knw", [HD, 1], F32, kind="ExternalInput")
    slp_d = nc.dram_tensor("slp", [P, HL], F32, kind="ExternalInput")
    nslp_d = nc.dram_tensor("nslp", [P, HL], F32, kind="ExternalInput")
    out_d = nc.dram_tensor("out", [s, DM], BF16, kind="ExternalOutput")

    with tile.TileContext(nc) as tc:
        with (
            tc.tile_pool(name="const", bufs=1) as const,
            tc.tile_pool(name="xt", bufs=2) as xt_pool,
            tc.tile_pool(name="big", bufs=1) as big,
            tc.tile_pool(name="sq", bufs=2) as sq_pool,
            tc.tile_pool(name="row1", bufs=3) as row1,
            tc.tile_pool(name="inv", bufs=2) as inv_pool,
            tc.tile_pool(name="nrow", bufs=2) as nr_pool,
            tc.tile_pool(name="jcol", bufs=2) as jc_pool,
            tc.tile_pool(name="tmp", bufs=4) as tmp_pool,
            tc.tile_pool(name="pt", bufs=4) as pt_pool,
            tc.tile_pool(name="fsb", bufs=3) as fsb_pool,
            tc.tile_pool(name="ps", bufs=8, space="PSUM") as ps,
        ):
            # ---------------- constants ----------------
            wq_sb = const.tile([P, ndm, HL * HD], BF16)
            wq_r = wq_d[:, :].rearrange("(o p) m -> p o m", p=P)
            nc.sync.dma_start(wq_sb[:, 0:ndm // 4, :], wq_r[:, 0:ndm // 4, :])
            wk_sb = const.tile([P, ndm, HD], BF16)
            nc.sync.dma_start(wk_sb, wk_d[:, :].rearrange("(o p) m -> p o m", p=P))
            wv_sb = const.tile([P, ndm, HD], BF16)
            wo_sb = const.tile([P, HL, DM], BF16)
            wo_r = wo_d[:, :].rearrange("(o p) m -> p o m", p=P)
            qnw_sb = const.tile([P, 1], F32)
            knw_sb = const.tile([P, 1], F32)
            slp_sb = const.tile([P, HL], F32)
            nslp_sb = const.tile([P, HL], F32)

            def deferred_const_loads():
                # Emitted after proj(0)'s first xt chunks: everything here is
                # first needed tens of microseconds into the kernel.
                for ch in range(1, 4):
                    o0 = ch * (ndm // 4)
                    nc.sync.dma_start(wq_sb[:, o0:o0 + ndm // 4, :],
                                      wq_r[:, o0:o0 + ndm // 4, :])
                nc.sync.dma_start(
                    wv_sb, wv_d[:, :].rearrange("(o p) m -> p o m", p=P))
                nc.sync.dma_start(qnw_sb, qnw_d[:, :])
                nc.sync.dma_start(knw_sb, knw_d[:, :])
                nc.sync.dma_start(slp_sb, slp_d[:, :])
                nc.sync.dma_start(nslp_sb, nslp_d[:, :])

            ones_f32 = const.tile([P, 1], F32)
            nc.vector.memset(ones_f32, 1.0)
            ones_sb = const.tile([P, 1], mybir.dt.float32r)
            nc.scalar.copy(ones_sb, ones_f32)
            ones_bf = const.tile([P, 1], BF16)
            nc.vector.memset(ones_bf, 1.0)
            eps_sb = const.tile([P, 1], F32)
            nc.vector.memset(eps_sb, EPS)

            # iota_row[p, f] = f ; iota_jcol[p, t] = 128*t + p
            iota_row = const.tile([P, 512], F32)
            nc.gpsimd.iota(iota_row, pattern=[[1, 512]], base=0,
                           channel_multiplier=0,
                           allow_small_or_imprecise_dtypes=True)
            iota_jcol = const.tile([P, njt], F32)
            nc.gpsimd.iota(iota_jcol, pattern=[[P, njt]], base=0,
                           channel_multiplier=1,
                           allow_small_or_imprecise_dtypes=True)

            # maskneg[p, f] = 0 where p <= f else -1e30  (additive causal
            # mask for diagonal 128x128 blocks of S^T)
            maskneg = const.tile([P, P], F32)
            nc.gpsimd.memset(maskneg, 0.0)
            nc.gpsimd.affine_select(
                out=maskneg, in_=maskneg,
                compare_op=ALU.is_ge, fill=NEG,
                base=0, pattern=[[1, P]], channel_multiplier=-1,
            )

            # ---------------- persistent activations ----------------
            # qt/ot hold only 2 query blocks (ring): the pipeline uses
            # qt of block ss right after proj(ss), and outproj consumes
            # ot of block ss-1 during attention of block ss.
            qt_sb = big.tile([P, HL, 2, 512], BF16)  # Q^T ring [d, h, ss%2, i]
            kt_sb = big.tile([P, s], BF16)           # K^T      [d, s]
            v_sb = big.tile([P, njt, HD], BF16)      # V        [s, d]
            ot_sb = big.tile([P, HL, 2, 512], BF16)  # O^T ring [d, h, ib%2, i]

            xt_r = xt_d[:, :].rearrange("(o p) t -> p o t", p=P)
            _loaded_consts = []

            def proj_slice(ss):
                s0 = ss * 512
                xt_t = xt_pool.tile([P, ndm, 512], BF16, name="xt_t")
                for ch in range(4):
                    o0 = ch * (ndm // 4)
                    nc.sync.dma_start(xt_t[:, o0:o0 + ndm // 4, :],
                                      xt_r[:, o0:o0 + ndm // 4, s0:s0 + 512])
                if ss == 0 and not _loaded_consts:
                    _loaded_consts.append(True)
                    deferred_const_loads()

                # Q (4 heads) and K projections: [d, s] (transposed)
                pq = [ps.tile([P, 512], F32, tag="ps", name=f"pq{c}")
                      for c in range(HL)]
                pk = ps.tile([P, 512], F32, tag="ps", name="pk")
                for o in range(ndm):
                    for c in range(HL):
                        nc.tensor.matmul(
                            pq[c], wq_sb[:, o, c * HD:(c + 1) * HD],
                            xt_t[:, o, :], start=(o == 0), stop=(o == ndm - 1))
                    nc.tensor.matmul(
                        pk, wk_sb[:, o, :], xt_t[:, o, :],
                        start=(o == 0), stop=(o == ndm - 1))

                # rmsnorm over d (partitions): ones-matmul sumsq -> rsqrt ->
                # partition_broadcast; fused scale+cast to bf16 on evict.
                for c in range(HL + 1):
                    src = pq[c] if c < HL else pk
                    w_sb = qnw_sb if c < HL else knw_sb
                    sqt = sq_pool.tile([P, 512], mybir.dt.float32r,
                                       tag="sq", name="sqt")
                    nc.scalar.activation(sqt, src, AF.Square)
                    psq = ps.tile([1, 512], F32, tag="ps", name="psq")
                    nc.tensor.matmul(psq, ones_sb, sqt,
                                     start=True, stop=True)
                    rms = row1.tile([1, 512], F32, tag="row1", name="rms")
                    nc.scalar.activation(rms, psq, AF.Sqrt,
                                         bias=eps_sb[:1, :], scale=1.0 / HD)
                    rec = row1.tile([1, 512], F32, tag="row1", name="rec")
                    nc.vector.reciprocal(rec, rms)
                    invb = inv_pool.tile([P, 512], F32, tag="inv", name="invb")
                    nc.gpsimd.partition_broadcast(invb, rec)
                    dst = qt_sb[:, c, ss % 2, :] if c < HL \
                        else kt_sb[:, s0:s0 + 512]
                    nc.vector.scalar_tensor_tensor(
                        out=dst, in0=src, scalar=w_sb, in1=invb,
                        op0=ALU.mult, op1=ALU.mult)

                # V projection: [s, d] natural
                for c in range(4):
                    pv = ps.tile([P, HD], F32, tag="ps", name="pv")
                    for o in range(ndm):
                        nc.tensor.matmul(
                            pv, xt_t[:, o, c * P:(c + 1) * P], wv_sb[:, o, :],
                            start=(o == 0), stop=(o == ndm - 1))
                    nc.scalar.copy(v_sb[:, 4 * ss + c, :], pv)

            def outproj_chunk(ib, mi_list):
                """Output projection for query block ib, m-slices mi_list."""
                for mi in mi_list:
                    m0 = mi * 512
                    for st_i in range(4):
                        s0 = ib * 512 + st_i * P
                        fps = ps.tile([P, 512], F32, tag="ps", name="fps")
                        for c in range(HL):
                            nc.tensor.matmul(
                                fps, ot_sb[:, c, ib % 2, st_i * P:(st_i + 1) * P],
                                wo_sb[:, c, m0:m0 + 512],
                                start=(c == 0), stop=(c == HL - 1))
                        fsb = fsb_pool.tile([P, 512], BF16, tag="fsb",
                                            name="fsb")
                        if (mi + st_i) % 2 == 0:
                            nc.scalar.copy(fsb, fps)
                        else:
                            nc.vector.tensor_copy(fsb, fps)
                        nc.sync.dma_start(out_d[s0:s0 + P, m0:m0 + 512], fsb)

            def attn_setup(ib, h):
                i0 = ib * 512
                negrow = nr_pool.tile([P, 512], F32, tag="nrow", name="negrow")
                nc.gpsimd.tensor_tensor(
                    negrow, iota_row,
                    nslp_sb[:, h:h + 1].to_broadcast([P, 512]), ALU.mult)
                # exp bias column: bias[p, jt] = slope * (128*jt + p - i0)
                jtmp = jc_pool.tile([P, njt], F32, tag="jcol", name="jtmp")
                nc.gpsimd.tensor_scalar_add(jtmp, iota_jcol, float(-i0))
                jcol = jc_pool.tile([P, njt], F32, tag="jcol", name="jcol")
                nc.gpsimd.tensor_tensor(
                    jcol, jtmp,
                    slp_sb[:, h:h + 1].to_broadcast([P, njt]), ALU.mult)
                otp = ps.tile([P, 512], F32, tag="ps", name="otp")
                lps = ps.tile([1, 512], F32, tag="ps", name="lps")
                return negrow, jcol, otp, lps

            def attn_jt(ib, h, jt, negrow, jcol, otp, lps):
                i0 = ib * 512
                jlast = 4 * (ib + 1) - 1
                j0 = jt * P
                c0 = max(0, j0 - i0)
                st = ps.tile([P, 512], F32, tag="ps", name="st")
                nc.tensor.matmul(
                    st[:, c0:], kt_sb[:, j0:j0 + P],
                    qt_sb[:, h, ib % 2, c0:],
                    start=True, stop=True)
                tmp = tmp_pool.tile([P, 512], F32, tag="tmp", name="tmp")
                nc.vector.tensor_tensor(
                    tmp[:, c0:], st[:, c0:], negrow[:, c0:], ALU.add)
                if j0 >= i0:  # diagonal block: additive causal mask
                    nc.gpsimd.tensor_tensor(
                        tmp[:, c0:c0 + P], tmp[:, c0:c0 + P],
                        maskneg, ALU.add)
                pt = pt_pool.tile([P, 512], BF16, tag="pt", name="pt")
                nc.scalar.activation(
                    pt[:, c0:], tmp[:, c0:], AF.Exp,
                    bias=jcol[:, jt:jt + 1], scale=1.0)
                # O^T accumulation: otp[d, i] += sum_j V[j, d] P^T[j, i]
                nc.tensor.matmul(
                    otp[:, c0:], v_sb[:, jt, :], pt[:, c0:],
                    start=(jt == 0), stop=(jt == jlast))
                # denominators: lps[0, i] += sum_j P^T[j, i]
                nc.tensor.matmul(
                    lps[:, c0:], ones_bf, pt[:, c0:],
                    start=(jt == 0), stop=(jt == jlast))

            def attn_finish(ib, h, otp, lps):
                lrow = row1.tile([1, 512], F32, tag="row1", name="lrow")
                nc.scalar.copy(lrow, lps)
                linv = row1.tile([1, 512], F32, tag="row1", name="linv")
                nc.vector.reciprocal(linv, lrow)
                linvb = inv_pool.tile([P, 512], F32, tag="inv", name="linvb")
                nc.gpsimd.partition_broadcast(linvb, linv)
                nc.vector.tensor_tensor(
                    ot_sb[:, h, ib % 2, :], otp, linvb, ALU.mult)

            def attn_head(ib, h):
                negrow, jcol, otp, lps = attn_setup(ib, h)
                for jt in range(4 * (ib + 1)):
                    attn_jt(ib, h, jt, negrow, jcol, otp, lps)
                attn_finish(ib, h, otp, lps)

            def attn_head_pair(ib, h0, h1):
                ctx0 = attn_setup(ib, h0)
                ctx1 = attn_setup(ib, h1)
                for jt in range(4 * (ib + 1)):
                    attn_jt(ib, h0, jt, *ctx0)
                    attn_jt(ib, h1, jt, *ctx1)
                attn_finish(ib, h0, ctx0[2], ctx0[3])
                attn_finish(ib, h1, ctx1[2], ctx1[3])

            for _rep in range(repeat):
                # pipelined: proj(ss) -> attention(ss) with outproj(ss-1)
                # interleaved at head granularity to fill PE bubbles.
                for ss in range(nss):
                    if 'proj' in phases:
                        proj_slice(ss)
                    if 'attn' in phases:
                        if ss == 0:
                            for h in range(HL):
                                attn_head(ss, h)
                                if _rep == 0:
                                    # Wo not needed until block 1: load it
                                    # in attention block 0's PE bubbles.
                                    m0 = h * (DM // 4)
                                    nc.sync.dma_start(
                                        wo_sb[:, :, m0:m0 + DM // 4],
                                        wo_r[:, :, m0:m0 + DM // 4])
                        else:
                            for h in range(HL):
                                attn_head(ss, h)
                                if 'out' in phases:
                                    outproj_chunk(ss - 1, [2 * h, 2 * h + 1])
                if 'out' in phases and 'attn' in phases:
                    outproj_chunk(nss - 1, list(range(8)))

    nc.finalize()
    return nc


def shard_inputs(x, Wq, Wk, Wv, Wo, q_norm_w, k_norm_w, s=S):
    """Host-side shard + layout prep. Returns per-core input maps."""
    slopes = _alibi_slopes(H)
    xt = np.ascontiguousarray(x.reshape(s, DM).T).astype(NBF)
    qnw = (np.asarray(q_norm_w, np.float32) / math.sqrt(HD)).reshape(HD, 1)
    knw = np.asarray(k_norm_w, np.float32).reshape(HD, 1).copy()
    in_maps = []
    for g in range(NC_CORES):
        qs = g * HL * HD
        sl = slopes[g * HL:(g + 1) * HL]
        in_maps.append({
            "xt": xt,
            "wq": np.ascontiguousarray(Wq[qs:qs + HL * HD, :].T).astype(NBF),
            "wk": np.ascontiguousarray(Wk[g * HD:(g + 1) * HD, :].T).astype(NBF),
            "wv": np.ascontiguousarray(Wv[g * HD:(g + 1) * HD, :].T).astype(NBF),
            "wo": np.ascontiguousarray(Wo[:, qs:qs + HL * HD].T).astype(NBF),
            "qnw": qnw,
            "knw": knw,
            "slp": np.ascontiguousarray(
                np.broadcast_to(sl, (P, HL))).astype(np.float32),
            "nslp": np.ascontiguousarray(
                np.broadcast_to(-sl, (P, HL))).astype(np.float32),
        })
    return in_maps


_MODULE_CACHE = {}
LAST_RESULT = None


def _get_module(s=S):
    if s not in _MODULE_CACHE:
        _MODULE_CACHE[s] = build_module(s)
    return _MODULE_CACHE[s]


def kernel(x, Wq, Wk, Wv, Wo, q_norm_w, k_norm_w, **run_kwargs):
    global LAST_RESULT
    from concourse.bass_utils import run_bass_kernel_spmd

    x = np.asarray(x)
    in_maps = shard_inputs(np.asarray(x), np.asarray(Wq), np.asarray(Wk),
                           np.asarray(Wv), np.asarray(Wo),
                           np.asarray(q_norm_w), np.asarray(k_norm_w))
    nc = _get_module(S)
    res = run_bass_kernel_spmd(nc, in_maps, core_ids=list(range(NC_CORES)),
                               **run_kwargs)
    LAST_RESULT = res
    acc = np.zeros((S, DM), np.float32)
    for r in res.results:
        acc += r["out"].astype(np.float32)
    return acc.reshape(B, S, DM)
